# revision 1
# baseline (speedup 1.0000x reference)
"""Trainium2 Bass kernel for CausalTensionGraphLayer.

Math (reference factorization):
  a   = x @ w1[:D] + b1         [T, H]   (H = D/2)
  c   = x @ w1[D:]              [T, H]
  vzb = x @ wv_w + wv_b         [T, D]
  hid_w  = silu(a[t] + c[t-w-1])               (c term is 0 when t-w-1 < 0)
  tau_w  = sigmoid(hid_w @ w2 + b2)
  msg[t] = sum_w tau_w[t] * vzb[t-w-1]         (vzb -> wv_b when t-w-1 < 0)
  y      = x @ merge_w[:D] + msg @ merge_w[D:] + merge_b
  out    = LayerNorm(y) * gamma + beta

Neighbor gathers are row shifts of x, so with zero rows prepended for the
out-of-range halo the same compute path reproduces the reference exactly
(zero x rows give c = 0 and vzb = wv_b).

Sharding: data-parallel over the B*T = 8192 token rows, 1024 own tokens per
core plus a 4-row halo (zeros at batch boundaries, neighbor rows otherwise).
No collectives. Host pre-casts x/weights to bf16 and pre-transposes x so the
device works feature-major (tokens on the free axis -> shifts are free-dim
offsets).

Schedule: phase A (a+c, all token quarters) needs only x/w1 so the PE starts
while wv and the merge weights are still streaming in; phase B (vzb) overlaps
the merge-weight loads; phases C (gating) and D (merge+LN) run per quarter.
Input DMAs are split across the two HWDGE queues (sync, scalar) in the exact
order the PE consumes them.
"""

from contextlib import ExitStack

import numpy as np
import ml_dtypes

import concourse.bass as bass
import concourse.bacc as bacc
import concourse.tile as tile
from concourse import mybir
from concourse.bass_utils import run_bass_kernel_spmd

BF16 = ml_dtypes.bfloat16

B, T, D = 2, 4096, 1024
H = D // 2
W = 4
EPS = 1e-5
NCORES = 8
NTOK = (B * T) // NCORES          # 1024 own tokens per core
HALO = W                          # 4
GRID = NTOK + HALO                # 1028 (halo + own)
NQ = 4                            # token quarters per core
QT = NTOK // NQ                   # 256 own tokens per quarter
QG = QT + HALO                    # 260: shifted-grid cols per quarter
KD = D // 128                     # 8 K-chunks over D
MH = H // 128                     # 4 M-tiles over H
MD = D // 128                     # 8 M-tiles over D
NT = QT // 128                    # 2 token tiles per quarter

FP32 = mybir.dt.float32
I32 = mybir.dt.int32
BF = mybir.dt.bfloat16
AF = mybir.ActivationFunctionType
ALU = mybir.AluOpType
AX = mybir.AxisListType


def build_nc(use_gamma_beta: bool, use_merge_b: bool):
    nc = bacc.Bacc(None, target_bir_lowering=False)

    xT = nc.dram_tensor("xT", [D, GRID], BF, kind="ExternalInput")
    w1a = nc.dram_tensor("w1a", [D, H], BF, kind="ExternalInput")
    w1c = nc.dram_tensor("w1c", [D, H], BF, kind="ExternalInput")
    wv = nc.dram_tensor("wv", [D, D], BF, kind="ExternalInput")
    m1 = nc.dram_tensor("m1", [D, D], BF, kind="ExternalInput")
    m2 = nc.dram_tensor("m2", [D, D], BF, kind="ExternalInput")
    w2rep = nc.dram_tensor("w2rep", [H, 128], BF, kind="ExternalInput")
    b1r = nc.dram_tensor("b1r", [128, MH], FP32, kind="ExternalInput")
    wvbr = nc.dram_tensor("wvbr", [128, MD], FP32, kind="ExternalInput")
    b2r = nc.dram_tensor("b2r", [128, 1], FP32, kind="ExternalInput")
    if use_gamma_beta:
        gam = nc.dram_tensor("gam", [1, D], FP32, kind="ExternalInput")
        bet = nc.dram_tensor("bet", [1, D], FP32, kind="ExternalInput")
    if use_merge_b:
        mbt = nc.dram_tensor("mbt", [1, D], FP32, kind="ExternalInput")
    y = nc.dram_tensor("y", [NTOK, D], FP32, kind="ExternalOutput")

    with tile.TileContext(nc) as tc, ExitStack() as ctx:
        persist = ctx.enter_context(tc.tile_pool(name="persist", bufs=1))
        abpool = ctx.enter_context(tc.tile_pool(name="abpool", bufs=NQ))
        qpool = ctx.enter_context(tc.tile_pool(name="qpool", bufs=2))
        mpool = ctx.enter_context(tc.tile_pool(name="mpool", bufs=4))
        mpool2 = ctx.enter_context(tc.tile_pool(name="mpool2", bufs=2))
        opool = ctx.enter_context(tc.tile_pool(name="opool", bufs=3))
        ps_acc = ctx.enter_context(tc.tile_pool(name="ps_acc", bufs=4, space="PSUM"))
        ps_log = ctx.enter_context(tc.tile_pool(name="ps_log", bufs=1, space="PSUM"))
        ps_y = ctx.enter_context(tc.tile_pool(name="ps_y", bufs=3, space="PSUM"))

        # ---- persistent loads, just-in-time order across both queues ----
        xT_sb = persist.tile([128, KD, GRID], BF, tag="xT")
        w1a_sb = persist.tile([128, KD, H], BF, tag="w1a")
        w1c_sb = persist.tile([128, KD, H], BF, tag="w1c")
        w2rep_sb = persist.tile([128, MH, 128], BF, tag="w2rep")
        wv_sb = persist.tile([128, KD, D], BF, tag="wv")
        m1_sb = persist.tile([128, KD, D], BF, tag="m1")
        m2_sb = persist.tile([128, KD, D], BF, tag="m2")
        b1_sb = persist.tile([128, MH], FP32, tag="b1")
        wvb_sb = persist.tile([128, MD], FP32, tag="wvb")
        b2_sb = persist.tile([128, 1], FP32, tag="b2")
        xT_r = xT.rearrange("(n p) t -> p n t", p=128)
        w1a_r = w1a.rearrange("(n p) m -> p n m", p=128)
        w1c_r = w1c.rearrange("(n p) m -> p n m", p=128)
        w2_r = w2rep.rearrange("(n p) m -> p n m", p=128)
        wv_r = wv.rearrange("(n p) m -> p n m", p=128)
        m1_r = m1.rearrange("(n p) m -> p n m", p=128)
        m2_r = m2.rearrange("(n p) m -> p n m", p=128)
        Q1 = QT + HALO
        # Greedy per-queue byte balancing in PE-consumption order:
        #   sync:   xTq0 | w1c | wv[0:4] | xTq3 | m1
        #   scalar: w1a | xTq1 | wv[4:8] | xTq2 | w2rep+biases | m2
        nc.sync.dma_start(out=xT_sb[:, :, 0:Q1], in_=xT_r[:, :, 0:Q1])
        for mc in range(MH):  # w1a col-chunks so a(q0, m=0) unblocks early
            nc.scalar.dma_start(
                out=w1a_sb[:, :, mc * 128:(mc + 1) * 128],
                in_=w1a_r[:, :, mc * 128:(mc + 1) * 128],
            )
        nc.scalar.dma_start(out=b1_sb, in_=b1r[:, :])
        for mc in range(MH):
            nc.sync.dma_start(
                out=w1c_sb[:, :, mc * 128:(mc + 1) * 128],
                in_=w1c_r[:, :, mc * 128:(mc + 1) * 128],
            )
        nc.scalar.dma_start(
            out=xT_sb[:, :, Q1:Q1 + QT], in_=xT_r[:, :, Q1:Q1 + QT]
        )
        nc.scalar.dma_start(out=wvb_sb, in_=wvbr[:, :])
        for mc in range(MD):
            eng = nc.sync if mc < 4 else nc.scalar
            eng.dma_start(
                out=wv_sb[:, :, mc * 128:(mc + 1) * 128],
                in_=wv_r[:, :, mc * 128:(mc + 1) * 128],
            )
        nc.scalar.dma_start(
            out=xT_sb[:, :, Q1 + QT:Q1 + 2 * QT],
            in_=xT_r[:, :, Q1 + QT:Q1 + 2 * QT],
        )
        nc.sync.dma_start(
            out=xT_sb[:, :, Q1 + 2 * QT:GRID], in_=xT_r[:, :, Q1 + 2 * QT:GRID]
        )
        nc.scalar.dma_start(out=w2rep_sb[:, :, :], in_=w2_r[:, :, :])
        nc.scalar.dma_start(out=b2_sb, in_=b2r[:, :])
        # merge weights last (first needed after phases A+B)
        nc.sync.dma_start(out=m1_sb[:, :, 0:512], in_=m1_r[:, :, 0:512])
        nc.scalar.dma_start(out=m2_sb[:, :, 0:512], in_=m2_r[:, :, 0:512])
        nc.sync.dma_start(out=m1_sb[:, :, 512:D], in_=m1_r[:, :, 512:D])
        nc.scalar.dma_start(out=m2_sb[:, :, 512:D], in_=m2_r[:, :, 512:D])
        magic_sb = persist.tile([128, 1], I32, tag="magic")
        nc.vector.memset(magic_sb, 0x5F3759DF)
        one_i = persist.tile([128, 1], I32, tag="onei")
        nc.vector.memset(one_i, 1)
        if use_gamma_beta:
            gam_sb = persist.tile([128, D], FP32, tag="gam")
            nc.sync.dma_start(out=gam_sb, in_=gam.partition_broadcast(128))
            bet_sb = persist.tile([128, D], FP32, tag="bet")
            nc.sync.dma_start(out=bet_sb, in_=bet.partition_broadcast(128))
        if use_merge_b:
            mb_sb = persist.tile([128, D], FP32, tag="mb")
            nc.sync.dma_start(out=mb_sb, in_=mbt.partition_broadcast(128))

        # ---- phase A: a (own grid) and c (shifted grid), all quarters ----
        aqs, cqs, vzqs = [], [], []
        for q in range(NQ):
            g0 = q * QT
            aq = abpool.tile([128, MH, QT], BF, tag="aq")
            aqs.append(aq)
            cq = abpool.tile([128, MH, QG], BF, tag="cq")
            cqs.append(cq)
            for m in range(MH):
                ps = ps_acc.tile([128, QT], FP32, tag="acc")
                for k in range(KD):
                    nc.tensor.matmul(
                        ps,
                        w1a_sb[:, k, m * 128:(m + 1) * 128],
                        xT_sb[:, k, g0 + HALO:g0 + HALO + QT],
                        start=(k == 0),
                        stop=(k == KD - 1),
                    )
                nc.scalar.activation(
                    out=aq[:, m, :], in_=ps, func=AF.Identity,
                    bias=b1_sb[:, m:m + 1], scale=1.0,
                )
            for m in range(MH):
                ps = ps_acc.tile([128, QG], FP32, tag="acc")
                for k in range(KD):
                    nc.tensor.matmul(
                        ps,
                        w1c_sb[:, k, m * 128:(m + 1) * 128],
                        xT_sb[:, k, g0:g0 + QG],
                        start=(k == 0),
                        stop=(k == KD - 1),
                    )
                nc.scalar.copy(out=cq[:, m, :], in_=ps)
        # ---- phase B: vzb (shifted grid), all quarters -------------------
        for q in range(NQ):
            g0 = q * QT
            vzq = abpool.tile([128, MD, QG], BF, tag="vzq")
            vzqs.append(vzq)
            for m in range(MD):
                ps = ps_acc.tile([128, QG], FP32, tag="acc")
                for k in range(KD):
                    nc.tensor.matmul(
                        ps,
                        wv_sb[:, k, m * 128:(m + 1) * 128],
                        xT_sb[:, k, g0:g0 + QG],
                        start=(k == 0),
                        stop=(k == KD - 1),
                    )
                nc.scalar.activation(
                    out=vzq[:, m, :], in_=ps, func=AF.Identity,
                    bias=wvb_sb[:, m:m + 1], scale=1.0,
                )
        # ---- phase C: gating (hid -> tau -> msg), per quarter ------------
        # silu(z) = z * sigmoid(z) keeps ScalarE in one activation-table set
        # for the whole kernel (silu/sqrt live in different sets; a switch
        # costs ~2.7us). tau comes out of its matmul pre-broadcast across
        # partitions because w2 is replicated over all 128 PE columns.
        msgqs = []
        for q in range(NQ):
            aq, cq, vzq = aqs[q], cqs[q], vzqs[q]
            tauq = qpool.tile([128, W, QT], BF, tag="tauq")
            for p in range(W // 2):
                hs = mpool2.tile([128, MH, 2, QT], BF, tag="hs")
                for wi in range(2):
                    w = 2 * p + wi
                    o = HALO - 1 - w
                    nc.vector.tensor_add(
                        hs[:, :, wi, :], aq, cq[:, :, o:o + QT]
                    )
                sg = mpool2.tile([128, MH, 2, QT], BF, tag="sg")
                nc.scalar.activation(out=sg, in_=hs, func=AF.Sigmoid)
                hss = mpool2.tile([128, MH, 2, QT], BF, tag="hids")
                nc.vector.tensor_mul(hss, hs, sg)
                pl = ps_log.tile([128, 2 * QT], FP32, tag="logit")
                for k in range(MH):
                    nc.tensor.matmul(
                        pl,
                        w2rep_sb[:, k, :],
                        hss[:, k, :, :],
                        start=(k == 0),
                        stop=(k == MH - 1),
                    )
                nc.scalar.activation(
                    out=tauq[:, 2 * p:2 * p + 2, :],
                    in_=pl.rearrange("p (a b) -> p a b", a=2),
                    func=AF.Sigmoid,
                    bias=b2_sb[:, 0:1], scale=1.0,
                )
            # msg = sum_w tau_w * shift(vzb, w+1): fused 3D bf16 ops with tau
            # broadcast over the 8 d-tiles via a step-0 mid dimension.
            msgq = qpool.tile([128, MD, QT], BF, tag="msgq")
            msgqs.append(msgq)

            def tau_b(w, tauq=tauq):
                s = tauq[:, w, :]
                return bass.AP(
                    tensor=s.tensor, offset=s.offset,
                    ap=[s.ap[0], [0, MD], s.ap[1]],
                )

            pw = []
            for w in range(W):
                o = HALO - 1 - w
                pt = mpool.tile([128, MD, QT], BF, tag="pw")
                nc.vector.tensor_mul(pt, tau_b(w), vzq[:, :, o:o + QT])
                pw.append(pt)
                if w == 1:
                    m01 = mpool.tile([128, MD, QT], BF, tag="pw")
                    nc.vector.tensor_add(m01, pw[0], pw[1])
            nc.vector.tensor_add(pw[3], pw[2], pw[3])
            nc.vector.tensor_add(msgq, m01, pw[3])
        # ---- phase D: merge + LayerNorm + store, per quarter -------------
        for q in range(NQ):
            g0 = q * QT
            msgq = msgqs[q]
            srow = mpool.tile([128, NT, 2], FP32, tag="srow")
            sqs = mpool.tile([128, NT, 2], FP32, tag="sqs")
            ysb = []
            for tt in range(NT):
                tok0 = g0 + 128 * tt
                yt = opool.tile([128, D], FP32, tag="ysb")
                ysb.append(yt)
                for half in range(2):
                    n0 = half * 512
                    yps = ps_y.tile([128, 512], FP32, tag="y")
                    for k in range(KD):
                        nc.tensor.matmul(
                            yps,
                            xT_sb[:, k, HALO + tok0:HALO + tok0 + 128],
                            m1_sb[:, k, n0:n0 + 512],
                            start=(k == 0),
                            stop=False,
                        )
                    for k in range(KD):
                        nc.tensor.matmul(
                            yps,
                            msgq[:, k, 128 * tt:128 * tt + 128],
                            m2_sb[:, k, n0:n0 + 512],
                            start=False,
                            stop=(k == KD - 1),
                        )
                    if use_merge_b:
                        nc.vector.tensor_add(yps, yps, mb_sb[:, n0:n0 + 512])
                    # Evict PSUM while collecting LN stats: Copy gives sum(y),
                    # Square gives sum(y^2) — both stay in the sigmoid table
                    # set. 'junk' is a write-only sink for the Square pass.
                    nc.scalar.activation(
                        out=yt[:, n0:n0 + 512], in_=yps, func=AF.Copy,
                        accum_out=srow[:, tt, half:half + 1],
                    )
                    junk = mpool2.tile([128, 512], FP32, tag="junk")
                    nc.scalar.activation(
                        out=junk, in_=yps, func=AF.Square,
                        accum_out=sqs[:, tt, half:half + 1],
                    )
            # LayerNorm finalize for both token tiles at once; rstd via
            # bit-trick seed + 2 Newton steps (keeps sqrt off ScalarE).
            ssum = mpool.tile([128, NT], FP32, tag="ssum")
            nc.vector.reduce_sum(out=ssum, in_=srow, axis=AX.X)
            qsum = mpool.tile([128, NT], FP32, tag="qsum")
            nc.vector.reduce_sum(out=qsum, in_=sqs, axis=AX.X)
            mean = mpool.tile([128, NT], FP32, tag="mean")
            nc.vector.tensor_scalar_mul(mean, ssum, 1.0 / D)
            m2e = mpool.tile([128, NT], FP32, tag="m2e")
            nc.vector.scalar_tensor_tensor(   # mean^2 - eps
                out=m2e, in0=mean, scalar=1.0, in1=mean,
                op0=ALU.mult, op1=ALU.mult,
            )
            nc.vector.tensor_scalar_add(m2e, m2e, -EPS)
            veps = mpool.tile([128, NT], FP32, tag="veps")
            nc.vector.scalar_tensor_tensor(   # q/D - (mean^2 - eps)
                out=veps, in0=qsum, scalar=1.0 / D, in1=m2e,
                op0=ALU.mult, op1=ALU.subtract,
            )
            rbits = mpool.tile([128, NT], I32, tag="rbits")
            nc.vector.tensor_scalar(
                out=rbits, in0=veps.bitcast(I32), scalar1=one_i[:, 0:1],
                scalar2=None, op0=ALU.arith_shift_right,
            )
            nc.vector.tensor_tensor(
                out=rbits, in0=magic_sb.to_broadcast([128, NT]), in1=rbits,
                op=ALU.subtract,
            )
            rstd = rbits.bitcast(FP32)
            for _ in range(2):
                nt1 = mpool.tile([128, NT], FP32, tag="nt1")
                nc.vector.tensor_mul(nt1, rstd, rstd)
                nc.vector.tensor_mul(nt1, nt1, veps)
                nc.vector.tensor_scalar(
                    out=nt1, in0=nt1, scalar1=-0.5, scalar2=1.5,
                    op0=ALU.mult, op1=ALU.add,
                )
                nc.vector.tensor_mul(rstd, rstd, nt1)
            for tt in range(NT):
                tok0 = g0 + 128 * tt
                nc.vector.tensor_scalar(
                    out=ysb[tt], in0=ysb[tt], scalar1=mean[:, tt:tt + 1],
                    scalar2=rstd[:, tt:tt + 1],
                    op0=ALU.subtract, op1=ALU.mult,
                )
                if use_gamma_beta:
                    nc.vector.tensor_mul(ysb[tt], ysb[tt], gam_sb)
                    nc.vector.tensor_add(ysb[tt], ysb[tt], bet_sb)
                nc.sync.dma_start(out=y[tok0:tok0 + 128, :], in_=ysb[tt])
    nc.compile()
    return nc


_CACHE: dict = {}


def _get_nc(use_gamma_beta: bool, use_merge_b: bool):
    key = (use_gamma_beta, use_merge_b)
    if key not in _CACHE:
        _CACHE[key] = build_nc(use_gamma_beta, use_merge_b)
    return _CACHE[key]


def kernel(x, w1, b1, w2, b2, wv_w, wv_b, merge_w, merge_b, gamma, beta):
    x = np.asarray(x, dtype=np.float32)
    w1 = np.asarray(w1, dtype=np.float32)
    b1 = np.asarray(b1, dtype=np.float32)
    w2 = np.asarray(w2, dtype=np.float32)
    b2 = np.asarray(b2, dtype=np.float32)
    wv_w = np.asarray(wv_w, dtype=np.float32)
    wv_b = np.asarray(wv_b, dtype=np.float32)
    merge_w = np.asarray(merge_w, dtype=np.float32)
    merge_b = np.asarray(merge_b, dtype=np.float32)
    gamma = np.asarray(gamma, dtype=np.float32)
    beta = np.asarray(beta, dtype=np.float32)

    use_gamma_beta = not (np.all(gamma == 1.0) and np.all(beta == 0.0))
    use_merge_b = bool(np.any(merge_b != 0.0))
    nc = _get_nc(use_gamma_beta, use_merge_b)

    x2 = x.reshape(B * T, D)
    shared = {
        "w1a": w1[:D].astype(BF16),
        "w1c": w1[D:].astype(BF16),
        "wv": wv_w.astype(BF16),
        "m1": merge_w[:D].astype(BF16),
        "m2": merge_w[D:].astype(BF16),
        "w2rep": np.ascontiguousarray(
            np.broadcast_to(w2.reshape(H, 1), (H, 128))
        ).astype(BF16),
        "b1r": np.ascontiguousarray(b1.reshape(MH, 128).T),
        "wvbr": np.ascontiguousarray(wv_b.reshape(MD, 128).T),
        "b2r": np.full((128, 1), float(b2[0]), np.float32),
    }
    if use_gamma_beta:
        shared["gam"] = gamma.reshape(1, D)
        shared["bet"] = beta.reshape(1, D)
    if use_merge_b:
        shared["mbt"] = merge_b.reshape(1, D)

    in_maps = []
    for c in range(NCORES):
        t0 = c * NTOK
        xs = np.zeros((GRID, D), np.float32)
        xs[HALO:] = x2[t0:t0 + NTOK]
        if t0 % T != 0:  # halo stays inside the same batch element
            xs[:HALO] = x2[t0 - HALO:t0]
        m = dict(shared)
        m["xT"] = np.ascontiguousarray(xs.T).astype(BF16)
        in_maps.append(m)

    res = run_bass_kernel_spmd(nc, in_maps, core_ids=list(range(NCORES)))
    out = np.concatenate([r["y"] for r in res.results], axis=0)
    return out.reshape(B, T, D).astype(np.float32)



# revision 5
# speedup vs baseline: 1.3960x; 1.3960x over previous
"""Trainium2 Bass kernel for CausalTensionGraphLayer.

Math (host-fused factorization):
  a   = x @ w1[:D] + b1                        [T, H]   (H = D/2)
  c   = x @ w1[D:]                             [T, H]
  hid_w  = silu(a[t] + c[t-w-1])               (c term is 0 when t-w-1 < 0)
  tau_w  = sigmoid(hid_w @ w2 + b2)
  u   = x @ (wv_w @ m2) + wv_b @ m2            [T, D]   (m2 = merge_w[D:])
  msg2[t] = sum_w tau_w[t] * u[t-w-1]          (== (msg @ m2)[t] by linearity;
                                                u -> wv_b @ m2 when t-w-1 < 0)
  y      = x @ merge_w[:D] + msg2 + merge_b
  out    = LayerNorm(y) * gamma + beta

Folding m2 into wv on the host removes a full [T,D]x[D,D] matmul from the
device: the value projection and the merge of the message happen in one
x @ Wc pass, and the feature-major msg2 is added into the token-major y
PSUM with cheap identity-matmul transposes (4 x N=128 per 512-wide tile).

Neighbor gathers are row shifts of x, so with zero rows prepended for the
out-of-range halo the same compute path reproduces the reference exactly.

Sharding: data-parallel over the B*T = 8192 token rows, 1024 own tokens per
core plus a 4-row halo (zeros at batch boundaries). No collectives.

All device inputs are pre-packed on the host into [128, bytes] partition-
major arrays so every DMA lands as 128 contiguous multi-KB descriptors
(the previous per-(k,m)-strided layout shredded loads into ~400B pieces and
left the PE waiting on weights for ~30us). Loads are split across the two
HWDGE trigger queues (sync, scalar) in PE consumption order.
"""

from contextlib import ExitStack

import numpy as np
import ml_dtypes

import concourse.bass as bass
import concourse.bacc as bacc
import concourse.tile as tile
from concourse import mybir
from concourse.bass_utils import run_bass_kernel_spmd

BF16 = ml_dtypes.bfloat16

B, T, D = 2, 4096, 1024
H = D // 2
W = 4
EPS = 1e-5
NCORES = 8
NTOK = (B * T) // NCORES          # 1024 own tokens per core
HALO = W                          # 4
NQ = 4                            # token quarters per core
QT = NTOK // NQ                   # 256 own tokens per quarter
QG = QT + HALO                    # 260 grid cols per quarter (4 halo + 256)
KD = D // 128                     # 8 K-chunks over D
MH = H // 128                     # 4 M-tiles over H
MD = D // 128                     # 8 M-tiles over D
NT = QT // 128                    # 2 token tiles per quarter

FP32 = mybir.dt.float32
I32 = mybir.dt.int32
BF = mybir.dt.bfloat16
AF = mybir.ActivationFunctionType
ALU = mybir.AluOpType
AX = mybir.AxisListType


def build_nc(use_gamma_beta: bool, use_merge_b: bool, use_b1: bool,
             use_bc: bool):
    nc = bacc.Bacc(None, target_bir_lowering=False)

    # Host-packed inputs: every tensor arrives as [128, free] with the
    # exact per-partition byte layout of its SBUF tile.
    xq_d = [nc.dram_tensor(f"xq{q}", [128, KD, QG], BF, kind="ExternalInput")
            for q in range(NQ)]
    w1a = nc.dram_tensor("w1a", [128, MH, KD, 128], BF, kind="ExternalInput")
    w1c = nc.dram_tensor("w1c", [128, MH, KD, 128], BF, kind="ExternalInput")
    wc = nc.dram_tensor("wc", [128, MD, KD, 128], BF, kind="ExternalInput")
    m1 = nc.dram_tensor("m1", [128, KD, D], BF, kind="ExternalInput")
    w2rep = nc.dram_tensor("w2rep", [128, MH, 128], BF, kind="ExternalInput")
    b2r = nc.dram_tensor("b2r", [128, 1], FP32, kind="ExternalInput")
    ident = nc.dram_tensor("ident", [128, 128], BF, kind="ExternalInput")
    if use_b1:
        b1r = nc.dram_tensor("b1r", [128, MH], FP32, kind="ExternalInput")
    if use_bc:
        bcr = nc.dram_tensor("bcr", [128, MD], FP32, kind="ExternalInput")
    if use_gamma_beta:
        gam = nc.dram_tensor("gam", [1, D], FP32, kind="ExternalInput")
        bet = nc.dram_tensor("bet", [1, D], FP32, kind="ExternalInput")
    if use_merge_b:
        mbt = nc.dram_tensor("mbt", [1, D], FP32, kind="ExternalInput")
    y = nc.dram_tensor("y", [NTOK, D], BF, kind="ExternalOutput")

    with tile.TileContext(nc) as tc, ExitStack() as ctx:
        persist = ctx.enter_context(tc.tile_pool(name="persist", bufs=1))
        abpool = ctx.enter_context(tc.tile_pool(name="abpool", bufs=NQ))
        qpool = ctx.enter_context(tc.tile_pool(name="qpool", bufs=2))
        mpool = ctx.enter_context(tc.tile_pool(name="mpool", bufs=4))
        mpool2 = ctx.enter_context(tc.tile_pool(name="mpool2", bufs=2))
        hpool = ctx.enter_context(tc.tile_pool(name="hpool", bufs=2 * NQ))
        opool = ctx.enter_context(tc.tile_pool(name="opool", bufs=3))
        ps_acc = ctx.enter_context(tc.tile_pool(name="ps_acc", bufs=3, space="PSUM"))
        ps_log = ctx.enter_context(tc.tile_pool(name="ps_log", bufs=1, space="PSUM"))
        ps_y = ctx.enter_context(tc.tile_pool(name="ps_y", bufs=4, space="PSUM"))

        # ---- persistent tiles ----
        xq_sb = [
            persist.tile([128, KD, QG], BF, tag=f"xq{q}", name=f"xq{q}")
            for q in range(NQ)
        ]
        w1a_sb = persist.tile([128, MH, KD, 128], BF, tag="w1a")
        w1c_sb = persist.tile([128, MH, KD, 128], BF, tag="w1c")
        wc_sb = persist.tile([128, MD, KD, 128], BF, tag="wc")
        m1_sb = persist.tile([128, KD, D], BF, tag="m1")
        w2rep_sb = persist.tile([128, MH, 128], BF, tag="w2rep")
        b2_sb = persist.tile([128, 1], FP32, tag="b2")
        id_sb = persist.tile([128, 128], BF, tag="ident")

        # ---- loads, just-in-time order across both trigger queues ----
        #   sync:   xq0[k<4] | w1c | xq2 | wc[m<4] | m1[k<4] | gam/bet/mb
        #   scalar: w1a(m0) | xq0[k>=4] | w1a(m1..3)+b1 | xq1 | w2+b2+ident
        #           | xq3 | wc[m>=4]+bc | m1[k>=4]
        nc.sync.dma_start(out=xq_sb[0][:, 0:4, :], in_=xq_d[0][:, 0:4, :])
        nc.scalar.dma_start(out=w1a_sb[:, 0, :, :], in_=w1a[:, 0, :, :])
        nc.scalar.dma_start(out=xq_sb[0][:, 4:KD, :], in_=xq_d[0][:, 4:KD, :])
        nc.scalar.dma_start(out=w1a_sb[:, 1:MH, :, :], in_=w1a[:, 1:MH, :, :])
        if use_b1:
            b1_sb = persist.tile([128, MH], FP32, tag="b1")
            nc.scalar.dma_start(out=b1_sb, in_=b1r[:, :])
        nc.sync.dma_start(out=w1c_sb[:, 0:2, :, :], in_=w1c[:, 0:2, :, :])
        nc.sync.dma_start(out=w1c_sb[:, 2:MH, :, :], in_=w1c[:, 2:MH, :, :])
        nc.scalar.dma_start(out=xq_sb[1][:, :, :], in_=xq_d[1][:, :, :])
        nc.scalar.dma_start(out=w2rep_sb[:, :, :], in_=w2rep[:, :, :])
        nc.scalar.dma_start(out=b2_sb, in_=b2r[:, :])
        nc.scalar.dma_start(out=id_sb, in_=ident[:, :])
        nc.sync.dma_start(out=xq_sb[2][:, :, :], in_=xq_d[2][:, :, :])
        nc.scalar.dma_start(out=xq_sb[3][:, :, :], in_=xq_d[3][:, :, :])
        nc.sync.dma_start(out=wc_sb[:, 0:4, :, :], in_=wc[:, 0:4, :, :])
        nc.scalar.dma_start(out=wc_sb[:, 4:MD, :, :], in_=wc[:, 4:MD, :, :])
        if use_bc:
            bc_sb = persist.tile([128, MD], FP32, tag="bc")
            nc.scalar.dma_start(out=bc_sb, in_=bcr[:, :])
        nc.sync.dma_start(out=m1_sb[:, 0:4, :], in_=m1[:, 0:4, :])
        nc.scalar.dma_start(out=m1_sb[:, 4:KD, :], in_=m1[:, 4:KD, :])
        magic_sb = persist.tile([128, 1], I32, tag="magic")
        nc.vector.memset(magic_sb, 0x5F3759DF)
        one_i = persist.tile([128, 1], I32, tag="onei")
        nc.vector.memset(one_i, 1)
        if use_gamma_beta:
            gam_sb = persist.tile([128, D], FP32, tag="gam")
            nc.sync.dma_start(out=gam_sb, in_=gam.partition_broadcast(128))
            bet_sb = persist.tile([128, D], FP32, tag="bet")
            nc.sync.dma_start(out=bet_sb, in_=bet.partition_broadcast(128))
        if use_merge_b:
            mb_sb = persist.tile([128, D], FP32, tag="mb")
            nc.sync.dma_start(out=mb_sb, in_=mbt.partition_broadcast(128))

        # ---- stage 1a: a (own tokens) and c (shifted grid) per quarter,
        #      with the gating elementwise prep trailing on DVE/ScalarE ----
        # CG = cols of c/u actually consumed by a quarter's gating windows.
        CG = QT + HALO - 1            # 259
        aqs, cqs, uqs, hsss = [], [], [], []
        for q in range(NQ):
            xs = xq_sb[q]
            aq = abpool.tile([128, MH, QT], BF, tag="aq")
            aqs.append(aq)
            cq = abpool.tile([128, MH, CG], BF, tag="cq")
            cqs.append(cq)
            for m in range(MH):
                ps = ps_acc.tile([128, QT], FP32, tag="acc")
                for k in range(KD):
                    nc.tensor.matmul(
                        ps, w1a_sb[:, m, k, :], xs[:, k, HALO:HALO + QT],
                        start=(k == 0), stop=(k == KD - 1),
                    )
                if use_b1:
                    nc.scalar.activation(
                        out=aq[:, m, :], in_=ps, func=AF.Identity,
                        bias=b1_sb[:, m:m + 1], scale=1.0,
                    )
                else:
                    nc.vector.tensor_scalar_mul(aq[:, m, :], ps, 1.0)
            for m in range(MH):
                ps = ps_acc.tile([128, CG], FP32, tag="acc")
                for k in range(KD):
                    nc.tensor.matmul(
                        ps, w1c_sb[:, m, k, :], xs[:, k, 0:CG],
                        start=(k == 0), stop=(k == KD - 1),
                    )
                nc.vector.tensor_scalar_mul(cq[:, m, :], ps, 1.0)
            # silu(z) = z * sigmoid(z): stays in the sigmoid table set.
            hq = []
            for p in range(W // 2):
                hs = mpool2.tile([128, MH, 2, QT], BF, tag="hs")
                for wi in range(2):
                    w = 2 * p + wi
                    o = HALO - 1 - w
                    nc.vector.tensor_add(hs[:, :, wi, :], aq, cq[:, :, o:o + QT])
                sg = mpool2.tile([128, MH, 2, QT], BF, tag="sg")
                nc.scalar.activation(out=sg, in_=hs, func=AF.Sigmoid)
                hss = hpool.tile([128, MH, 2, QT], BF, tag="hss")
                nc.vector.tensor_mul(hss, hs, sg)
                hq.append(hss)
            hsss.append(hq)
        # ---- stage 1b: u = x @ Wc (+ wv_b @ m2), shifted grid ----
        for q in range(NQ):
            xs = xq_sb[q]
            uq = abpool.tile([128, MD, CG], BF, tag="uq")
            uqs.append(uq)
            for m in range(MD):
                ps = ps_acc.tile([128, CG], FP32, tag="acc")
                for k in range(KD):
                    nc.tensor.matmul(
                        ps, wc_sb[:, m, k, :], xs[:, k, 0:CG],
                        start=(k == 0), stop=(k == KD - 1),
                    )
                if use_bc:
                    nc.scalar.activation(
                        out=uq[:, m, :], in_=ps, func=AF.Identity,
                        bias=bc_sb[:, m:m + 1], scale=1.0,
                    )
                else:
                    nc.vector.tensor_scalar_mul(uq[:, m, :], ps, 1.0)
        # ---- stage 2: per quarter gate -> msg2 -> y -> LN -> store ----
        for q in range(NQ):
            aq, cq, uq = aqs[q], cqs[q], uqs[q]
            tauq = qpool.tile([128, W, QT], BF, tag="tauq")
            for p in range(W // 2):
                hss = hsss[q][p]
                pl = ps_log.tile([128, 2 * QT], FP32, tag="logit")
                for k in range(MH):
                    nc.tensor.matmul(
                        pl, w2rep_sb[:, k, :], hss[:, k, :, :],
                        start=(k == 0), stop=(k == MH - 1),
                    )
                # tau pre-broadcast across partitions (w2 replicated cols).
                nc.scalar.activation(
                    out=tauq[:, 2 * p:2 * p + 2, :],
                    in_=pl.rearrange("p (a b) -> p a b", a=2),
                    func=AF.Sigmoid, bias=b2_sb[:, 0:1], scale=1.0,
                )
            # msg2 = sum_w tau_w * shift(u, w+1): tau broadcast over the
            # 8 d-tiles via a step-0 mid dimension.
            msgq = qpool.tile([128, MD, QT], BF, tag="msgq")

            def tau_b(w, tauq=tauq):
                s = tauq[:, w, :]
                return bass.AP(
                    tensor=s.tensor, offset=s.offset,
                    ap=[s.ap[0], [0, MD], s.ap[1]],
                )

            pw = []
            for w in range(W):
                o = HALO - 1 - w
                pt = mpool.tile([128, MD, QT], BF, tag="pw")
                nc.vector.tensor_mul(pt, tau_b(w), uq[:, :, o:o + QT])
                pw.append(pt)
                if w == 1:
                    m01 = mpool.tile([128, MD, QT], BF, tag="pw")
                    nc.vector.tensor_add(m01, pw[0], pw[1])
            nc.vector.tensor_add(pw[3], pw[2], pw[3])
            nc.vector.tensor_add(msgq, m01, pw[3])
            # y = x @ m1 + msg2^T (+ merge_b), token-major via PSUM:
            # identity-rhs matmuls transpose-accumulate msg2 into the m1 sum.
            g0 = q * QT
            srow = mpool.tile([128, NT, 2], FP32, tag="srow")
            sqs = mpool.tile([128, NT, 2], FP32, tag="sqs")
            ysb = []
            for tt in range(NT):
                yt = opool.tile([128, D], FP32, tag="ysb")
                ysb.append(yt)
                for half in range(2):
                    n0 = half * 512
                    yps = ps_y.tile([128, 512], FP32, tag="y")
                    for k in range(KD):
                        nc.tensor.matmul(
                            yps,
                            xq_sb[q][:, k, HALO + 128 * tt:HALO + 128 * tt + 128],
                            m1_sb[:, k, n0:n0 + 512],
                            start=(k == 0), stop=False,
                        )
                    for j in range(4):
                        f = 4 * half + j
                        nc.tensor.matmul(
                            yps[:, 128 * j:128 * j + 128],
                            msgq[:, f, 128 * tt:128 * tt + 128],
                            id_sb,
                            start=False, stop=True,
                        )
                    if use_merge_b:
                        nc.vector.tensor_add(yps, yps, mb_sb[:, n0:n0 + 512])
                    # Evict PSUM while collecting LN stats: Copy gives
                    # sum(y), Square gives sum(y^2); both in sigmoid set.
                    nc.scalar.activation(
                        out=yt[:, n0:n0 + 512], in_=yps, func=AF.Copy,
                        accum_out=srow[:, tt, half:half + 1],
                    )
                    junk = mpool2.tile([128, 512], FP32, tag="junk")
                    nc.scalar.activation(
                        out=junk, in_=yps, func=AF.Square,
                        accum_out=sqs[:, tt, half:half + 1],
                    )
            # LayerNorm finalize for both token tiles at once; rstd via
            # bit-trick seed + 2 Newton steps (keeps sqrt off ScalarE).
            ssum = mpool.tile([128, NT], FP32, tag="ssum")
            nc.vector.reduce_sum(out=ssum, in_=srow, axis=AX.X)
            qsum = mpool.tile([128, NT], FP32, tag="qsum")
            nc.vector.reduce_sum(out=qsum, in_=sqs, axis=AX.X)
            mean = mpool.tile([128, NT], FP32, tag="mean")
            nc.vector.tensor_scalar_mul(mean, ssum, 1.0 / D)
            m2e = mpool.tile([128, NT], FP32, tag="m2e")
            nc.vector.scalar_tensor_tensor(   # mean^2 - eps
                out=m2e, in0=mean, scalar=1.0, in1=mean,
                op0=ALU.mult, op1=ALU.mult,
            )
            nc.vector.tensor_scalar_add(m2e, m2e, -EPS)
            veps = mpool.tile([128, NT], FP32, tag="veps")
            nc.vector.scalar_tensor_tensor(   # q/D - (mean^2 - eps)
                out=veps, in0=qsum, scalar=1.0 / D, in1=m2e,
                op0=ALU.mult, op1=ALU.subtract,
            )
            rbits = mpool.tile([128, NT], I32, tag="rbits")
            nc.vector.tensor_scalar(
                out=rbits, in0=veps.bitcast(I32), scalar1=one_i[:, 0:1],
                scalar2=None, op0=ALU.arith_shift_right,
            )
            nc.vector.tensor_tensor(
                out=rbits, in0=magic_sb.to_broadcast([128, NT]), in1=rbits,
                op=ALU.subtract,
            )
            rstd = rbits.bitcast(FP32)
            for _ in range(2):
                nt1 = mpool.tile([128, NT], FP32, tag="nt1")
                nc.vector.tensor_mul(nt1, rstd, rstd)
                nc.vector.tensor_mul(nt1, nt1, veps)
                nc.vector.tensor_scalar(
                    out=nt1, in0=nt1, scalar1=-0.5, scalar2=1.5,
                    op0=ALU.mult, op1=ALU.add,
                )
                nc.vector.tensor_mul(rstd, rstd, nt1)
            for tt in range(NT):
                tok0 = g0 + 128 * tt
                yo = opool.tile([128, D], BF, tag="yout")
                nc.vector.tensor_scalar(
                    out=yo, in0=ysb[tt], scalar1=mean[:, tt:tt + 1],
                    scalar2=rstd[:, tt:tt + 1],
                    op0=ALU.subtract, op1=ALU.mult,
                )
                if use_gamma_beta:
                    nc.vector.tensor_mul(yo, yo, gam_sb)
                    nc.vector.tensor_add(yo, yo, bet_sb)
                nc.sync.dma_start(out=y[tok0:tok0 + 128, :], in_=yo)
    nc.compile()
    return nc


_CACHE: dict = {}


def _get_nc(flags):
    if flags not in _CACHE:
        _CACHE[flags] = build_nc(*flags)
    return _CACHE[flags]


def _pack_km(wmat, mt):
    """[D, mt*128] weight -> [128, mt, KD, 128] m-outer partition-major."""
    # w[k*128+p, m*128+c] -> out[p, m, k, c]
    wr = wmat.reshape(KD, 128, mt, 128)
    return np.ascontiguousarray(wr.transpose(1, 2, 0, 3)).astype(BF16)


def kernel(x, w1, b1, w2, b2, wv_w, wv_b, merge_w, merge_b, gamma, beta):
    x = np.asarray(x, dtype=np.float32)
    w1 = np.asarray(w1, dtype=np.float32)
    b1 = np.asarray(b1, dtype=np.float32)
    w2 = np.asarray(w2, dtype=np.float32)
    b2 = np.asarray(b2, dtype=np.float32)
    wv_w = np.asarray(wv_w, dtype=np.float32)
    wv_b = np.asarray(wv_b, dtype=np.float32)
    merge_w = np.asarray(merge_w, dtype=np.float32)
    merge_b = np.asarray(merge_b, dtype=np.float32)
    gamma = np.asarray(gamma, dtype=np.float32)
    beta = np.asarray(beta, dtype=np.float32)

    m2w = merge_w[D:]                       # [D, D]
    wc = wv_w @ m2w                         # fused value+merge projection
    bc = wv_b @ m2w                         # [D]
    use_gamma_beta = not (np.all(gamma == 1.0) and np.all(beta == 0.0))
    use_merge_b = bool(np.any(merge_b != 0.0))
    use_b1 = bool(np.any(b1 != 0.0))
    use_bc = bool(np.any(bc != 0.0))
    flags = (use_gamma_beta, use_merge_b, use_b1, use_bc)
    nc = _get_nc(flags)

    x2 = x.reshape(B * T, D)
    shared = {
        "w1a": _pack_km(w1[:D], MH),
        "w1c": _pack_km(w1[D:], MH),
        "wc": _pack_km(wc, MD),
        "m1": np.ascontiguousarray(
            merge_w[:D].reshape(KD, 128, D).transpose(1, 0, 2)
        ).astype(BF16),
        "w2rep": np.ascontiguousarray(
            np.broadcast_to(w2.reshape(MH, 128, 1), (MH, 128, 128))
            .transpose(1, 0, 2)
        ).astype(BF16),
        "b2r": np.full((128, 1), float(b2[0]), np.float32),
        "ident": np.eye(128, dtype=BF16),
    }
    if use_b1:
        shared["b1r"] = np.ascontiguousarray(b1.reshape(MH, 128).T)
    if use_bc:
        shared["bcr"] = np.ascontiguousarray(bc.reshape(MD, 128).T)
    if use_gamma_beta:
        shared["gam"] = gamma.reshape(1, D)
        shared["bet"] = beta.reshape(1, D)
    if use_merge_b:
        shared["mbt"] = merge_b.reshape(1, D)

    in_maps = []
    for c in range(NCORES):
        t0 = c * NTOK
        xs = np.zeros((NTOK + HALO, D), np.float32)
        xs[HALO:] = x2[t0:t0 + NTOK]
        if t0 % T != 0:  # halo stays inside the same batch element
            xs[:HALO] = x2[t0 - HALO:t0]
        xsT = xs.T.reshape(KD, 128, NTOK + HALO)  # [k, p, grid]
        m = dict(shared)
        for q in range(NQ):
            g0 = q * QT
            m[f"xq{q}"] = np.ascontiguousarray(
                xsT[:, :, g0:g0 + QG].transpose(1, 0, 2)
            ).astype(BF16)
        in_maps.append(m)

    res = run_bass_kernel_spmd(nc, in_maps, core_ids=list(range(NCORES)))
    out = np.concatenate(
        [np.asarray(r["y"]).astype(np.float32) for r in res.results], axis=0
    )
    return out.reshape(B, T, D)
